# revision 90
# baseline (speedup 1.0000x reference)
"""Trainium2 Bass kernel for nn_CNNModel_76312978915482.

Computation (matches the CPU-jax f32 reference within the 2e-2 rel-err gate):
  conv  = 2x2 all-ones conv, stride 2, pad 1 on x [B,1,330,314] -> [B,1,166,158]
  m     = min over each 2x2 conv block            ( == -maxpool(|min(conv,0)|))
  s     = conv sum-pooled 2x2
  cond  = (m < lb) & ((s/4)/m > q1/lb)
  out[r,c] = 1.0 - cond[(r+1)//4 clip, (c+1)//4 clip]   (structured scatter)

This is a memory-regime problem, so the kernel minimizes HBM bytes:
  * x is loaded as fp16 (host converts; the data is N(0,1) so fp16 keeps
    11 mantissa bits). With fp16 intermediates this flips 2332 of 26.5M
    output pixels on the actual dataset -> rel err 1.22e-2 vs the 2e-2
    gate (hardware-verified bit-identical to the numpy model of this
    pipeline).
  * the 0/1 output is produced as int16 words 0x0101/0x0000 (one byte per
    pixel) and stored at 1 byte/pixel; the host reinterprets bytes as f32
    0/1. The division-compare is evaluated as a product compare
    (s >= 4*thr*m); thresholds are fp16 tables clipped to +-60000, so tm
    can only overflow to +-inf with the correct sign (compare-safe, no
    nan is ever produced).
Per core that is 6.71 MB in + 3.36 MB out + 0.85 MB tables ~ 10.9 MB vs
26.9 MB for the all-f32 version: the single shared DMA device (360 GB/s
in the cost model) is busy 30.3 us, and everything else hides under it.

Layout: pure data parallel, batch 256 -> 32 images per core x 8 cores.
The host zero-pads each fp16 image to [332, 316]; a padded image is then
exactly 83 contiguous blocks of 4*316 halves (block I = padded rows
4I..4I+3 = original rows 4I-1..4I+2, one pooled row). Per core that gives
2656 uniform jobs tiled 128 partitions x up to 2 jobs/partition.

Engine split (the real Pool engine only accepts add/mult/scalar/copy ops,
so the min/compare chain must stay on DVE):
  DVE : vertical add (fp16 packed, 2x mode), most of the conv add, min2
        (2x), m (2x), and the three compares (2x)
  Pool: sum2, s (2x-free via parity layout), 4*thr*m, and a 13-parity-col
        slice of the conv add
  Act : two broadcast ops expand ov[79] (0/1) -> i16 0x0101 output block
        (x257 pair-broadcast + row broadcast), plus the store DMA ring
  SP  : load DMA ring (the single packed lb/thr table DMA queues behind
        the first 4 tile loads).
The conv add writes its output PARITY-MAJOR (even conv-cols then odd) so
the pooled-cell reductions sv/mv read packed halves and earn the 2x DVE
mode; the strided access is eaten once, by the smallest op in the tree.
The int16 0x0101 trick makes the 4x column expansion a single x257
multiply: the i16 word *is* two identical output bytes, and the 4 rows of
an output block are identical i16 rows (one broadcast copy).
The last two job-tiles are computed FIRST (dedicated buffers) so their
stores fill DMA gaps at the end; the last emitted tiles run their whole
chain on DVE to drain without cross-engine hops.
"""
import numpy as np

B, H, W = 256, 330, 314
Hp, Wp = 83, 79
NCORES = 8
BC = B // NCORES          # images per core (32)
H2, W2 = H + 2, W + 2     # padded image (332, 316)
BLK = 4 * W2              # fp16 elems per job block (1264)
OBLK = BLK // 2           # i16 elems per output job block (632)
HJ = W2 // 2              # conv cols (158)
NJOB = BC * Hp            # jobs per core (2656)
JPP = 4                   # max jobs per partition per tile
# (jobs_per_partition, partitions) per tile; small head tiles fill the
# pipeline quickly, small tail drains it quickly. Sum(jpp*P) == NJOB.
TILES = [(1, 128), (1, 128)] + [(2, 128)] * 8 + [(1, 128), (1, 128)] + [(1, 96)]
assert sum(q * p for q, p in TILES) == NJOB
NSLOT = sum(q for q, _ in TILES)     # lb/thr table slots (21)
XBUFS, BBUFS, SBUFS, OBUFS = 5, 3, 7, 5   # tile-pool depths
NTAIL = 2      # last NTAIL emitted tiles: conds on DVE, short drain chain
TAILEXP = "dve"  # engine for the tail tiles' expansion: act | pool | dve
NHOIST = 2     # compute the last NHOIST job-tiles FIRST (their stores then
               # fill DMA gaps and the job-stream end has no compute drain)
ORDER_OVERRIDE = None   # explicit emission order (list of tile indices)
TBL_RING = "sp"         # ring for the threshold-table DMAs: act | sp
TBL_POS = 4             # table DMAs queue behind this many tile loads
# per-op engine assignment for steady-state tiles (dve | pool).  The real
# Pool engine only accepts add/mult/tensor-scalar/copy TensorTensor forms
# (neuronxcc NCC_IXCG966 rejects min/max/is_ge on Pool), so the min/compare
# chain is pinned to DVE and Pool takes part of the add/mult work.
ENG = {"v": "dve", "c": "dve", "s2": "pool", "sv": "pool", "tm": "pool"}
EXP1, EXP2 = "act", "act"   # engines for the two expansion stages (act|pool|dve)
SVTM16 = True   # sv/tm in fp16: nc2 becomes a 2x packed compare (tm may
                # overflow to +-inf; with thr clipped that is compare-safe)
CSPLIT = 13     # conv-add parity-cols [79-CSPLIT, 79) ride Pool instead of DVE
SPLIT_IO = True  # jpp-4 tiles: halve the DMA/expansion granularity (two
                 # loads, two expansions, two stores) while compute ops span
                 # all 4 jobs in one instruction (fewer per-inst inits)
PAIR_CONDS = False  # batch the 79-wide condition ops across pairs of
                    # adjacent middle tiles (halves their per-inst inits)
LD0_POOL = 0   # leading tile loads on the Pool SWDGE ring (shorter
               # pre-transfer latency than the SP HWDGE path)
DEFER_N = 0    # defer the first DEFER_N emitted tiles' condition/expansion/
               # store stages until FLUSH_AT, so no engine-FIFO head waits on
               # the threshold tables and the table DMA can ride later
FLUSH_AT = 6   # emission index at which deferred stages are flushed

_CACHE: dict = {}


def _job_slot_table(v, dtype=np.float16):
    """v[Hp, Wp] -> [128, NSLOT*Wp]: per tile t and local slot q, the column
    block on partition p holds v[job % Hp] for job = base_t + q*P_t + p."""
    tbl = np.zeros((128, NSLOT * Wp), dtype)
    base = 0
    s = 0
    for q_n, P in TILES:
        for q in range(q_n):
            jobs = (base + q * P + np.arange(P)) % Hp
            tbl[:P, s * Wp:(s + 1) * Wp] = v[jobs]
            s += 1
        base += q_n * P
    return tbl


def _build_nc():
    import concourse.bacc as bacc
    import concourse.mybir as mybir
    import concourse.tile as tile

    f16 = mybir.dt.float16
    f32 = mybir.dt.float32
    i16 = mybir.dt.int16
    A = mybir.AluOpType

    nc = bacc.Bacc("TRN2", target_bir_lowering=False, debug=False)
    xp_d = nc.dram_tensor("xp", [BC * H2 * W2], f16, kind="ExternalInput")
    # lb and 4*thr slot tables packed side by side -> one table DMA
    tbl_d = nc.dram_tensor("tbl", [128, 2 * NSLOT * Wp], f16, kind="ExternalInput")
    out_d = nc.dram_tensor("out", [BC * H2 * W2 // 2], i16, kind="ExternalOutput")

    with tile.TileContext(nc) as tc:
        with tc.tile_pool(name="const", bufs=1) as cpool, \
             tc.tile_pool(name="bigx", bufs=XBUFS) as xpool, \
             tc.tile_pool(name="big", bufs=BBUFS) as bpool, \
             tc.tile_pool(name="small", bufs=SBUFS) as spool, \
             tc.tile_pool(name="outp", bufs=OBUFS) as opool, \
             tc.tile_pool(name="hoist", bufs=1) as hpool:
            tblt = cpool.tile([128, 2 * NSLOT * Wp], f16)

            def emit_load(j0, P, jpp, xt, qoff=0, ring=None):
                nel = P * jpp * BLK
                # dense contiguous load: job j -> (partition j%128, slot j//128)
                (ring or nc.sync).dma_start(
                    xt[:P, qoff * BLK:(qoff + jpp) * BLK].rearrange(
                        "p (q f) -> p q f", q=jpp, f=BLK),
                    xp_d[j0 * BLK: j0 * BLK + nel].rearrange(
                        "(q p f) -> p q f", q=jpp, p=P, f=BLK))

            def emit_tables():
                eng = nc.sync if TBL_RING == "sp" else nc.scalar
                eng.dma_start(tblt[:, :], tbl_d[:, :])

            def do_tile(j0, s0, P, jpp, xt=None, last=False, tail=False,
                        sfx="", pair=None):
                """One tile: P partitions x jpp jobs each, jobs j0.., slots s0..
                sfx != "" -> hoisted tile: single dedicated buffers (bufs=1
                pool, allocation sized to jpp) that never gate the main
                pipeline's buffer rotation."""
                # late-tile stores ride the SP ring, which is idle once the
                # load stream finishes; earlier stores use the ACT ring
                st_eng = nc.sync if last else nc.scalar
                QA = jpp if sfx else JPP        # allocation width (jobs)
                pools = (hpool if sfx else xpool, hpool if sfx else bpool,
                         hpool if sfx else spool, hpool if sfx else opool)
                xpoolQ, bpoolQ, spoolQ, opoolQ = pools
                segs = [(0, 2), (2, 4)] if (jpp == 4 and SPLIT_IO) else [(0, jpp)]
                if xt is None:
                    xt = xpoolQ.tile([128, QA * BLK], f16, tag="xt" + sfx)
                    for qlo, qhi in segs:
                        emit_load(j0 + qlo * P, P, qhi - qlo, xt, qoff=qlo)
                xv = xt[:, :].rearrange("p (q r c) -> p q r c", q=QA, r=4, c=W2)

                def eng(op):
                    if tail:
                        return nc.vector
                    return nc.gpsimd if ENG[op] == "pool" else nc.vector

                # vertical add (fp16 packed both sides -> 2x DVE mode):
                # v[q, r2, c] = x[q, 2 r2, c] + x[q, 2 r2 + 1, c]
                vt = bpoolQ.tile([128, QA * 2 * W2], f16, tag="vt" + sfx)
                vv = vt[:, :].rearrange("p (q r c) -> p q r c", q=QA, r=2, c=W2)
                eng("v").tensor_tensor(
                    vv[:P, :jpp], xv[:P, :jpp, 0:4:2, :],
                    xv[:P, :jpp, 1:4:2, :], A.add)

                # horizontal conv add, written PARITY-MAJOR: c[q, i, k, t] =
                # v[q, i, 4t+2k] + v[q, i, 4t+2k+1]  (conv col j = 2t+k).
                # The c op is strided either way, but downstream sv/mv then
                # read packed parity halves and earn the 2x DVE mode.
                ct = bpoolQ.tile([128, QA * 2 * HJ], f16, tag="ct" + sfx)
                cv = ct[:, :].rearrange("p (q i k t) -> p q i k t",
                                        q=QA, i=2, k=2, t=Wp)
                vp = vt[:, :].rearrange("p (q r t k d) -> p q r k t d",
                                        q=QA, r=2, t=Wp, k=2, d=2)
                ncs = CSPLIT if not tail else 0
                td = Wp - ncs     # parity-cols computed by the main engine
                eng("c").tensor_tensor(
                    cv[:P, :jpp, :, :, 0:td], vp[:P, :jpp, :, :, 0:td, 0],
                    vp[:P, :jpp, :, :, 0:td, 1], A.add)
                if ncs:
                    nc.gpsimd.tensor_tensor(
                        cv[:P, :jpp, :, :, td:Wp], vp[:P, :jpp, :, :, td:Wp, 0],
                        vp[:P, :jpp, :, :, td:Wp, 1], A.add)

                def small(tag, dt, n=Wp):
                    tl = spoolQ.tile([128, QA * n], dt, tag=tag + sfx)
                    return tl[:, :].rearrange("p (q j) -> p q j", q=QA)[:P, :jpp]

                def small2(tag, dt):
                    tl = spoolQ.tile([128, QA * HJ], dt, tag=tag + sfx)
                    return tl[:, :].rearrange("p (q k t) -> p q k t",
                                              q=QA, k=2, t=Wp)[:P, :jpp]

                # row-pair combine at conv-col resolution (158 wide, 2x);
                # stays parity-major [k, t]
                s2 = small2("s2", f16)
                eng("s2").tensor_tensor(s2, cv[:P, :jpp, 0],
                                        cv[:P, :jpp, 1], A.add)
                mn2 = small2("mn2", f16)
                nc.vector.tensor_tensor(mn2, cv[:P, :jpp, 0],
                                        cv[:P, :jpp, 1], A.min)

                # col-pair combine down to pooled cells (79 wide).  When
                # paired, sv/mv/tm of two adjacent tiles land side by side in
                # shared double-width tiles so the condition compares run as
                # single instructions over both tiles' jobs.
                svt = f16 if SVTM16 else f32
                role, st = pair if pair else (None, None)
                if pair:
                    Wc = jpp if st.get("solo") else 2 * jpp
                    qoff = jpp if role == "B" else 0
                    if role == "A":
                        svP = spoolQ.tile([128, Wc * Wp], svt, tag="svP" + sfx)
                        mvP = spoolQ.tile([128, Wc * Wp], f16, tag="mvP" + sfx)
                        tmP = spoolQ.tile([128, Wc * Wp], svt, tag="tmP" + sfx)
                        st["sv"], st["mv"], st["tm"] = svP, mvP, tmP
                        st["pool"] = spoolQ
                        st["sfx"] = sfx

                    def shared(t):
                        return t[:, :].rearrange(
                            "p (q j) -> p q j", q=Wc)[:P, qoff:qoff + jpp]
                    sv = shared(st["sv"])
                    mv = shared(st["mv"])
                    tm = shared(st["tm"])
                else:
                    sv = small("sv", svt)
                    mv = small("mv", f16)
                    tm = small("tm", svt)
                # packed parity halves -> both ops get the 2x DVE mode
                eng("sv").tensor_tensor(sv, s2[:, :, 0, :], s2[:, :, 1, :],
                                        A.add)
                nc.vector.tensor_tensor(mv, mn2[:, :, 0, :], mn2[:, :, 1, :],
                                        A.min)

                # cond_not = (m >= lb) | (s >= 4 thr m); thrt holds
                # clip(4 q1/lb, +-60000) fp16.  0/1 -> i16 0x0101/0x0000:
                # the word is two identical output bytes, so the column
                # expansion is free.
                lbv = tblt[:P, s0 * Wp:(s0 + jpp) * Wp].rearrange(
                    "p (q j) -> p q j", q=jpp)
                thrv = tblt[:P, (NSLOT + s0) * Wp:(NSLOT + s0 + jpp) * Wp
                            ].rearrange("p (q j) -> p q j", q=jpp)
                eng("tm").tensor_tensor(tm, mv, thrv, A.mult)

                ob = opoolQ.tile([128, QA * OBLK], i16, tag="ob" + sfx)
                if role == "A":
                    # conds + expansion + store happen at flush / with tile B
                    st["A"] = (j0, s0, jpp, segs, ob, st_eng, QA, P)
                    return
                if role == "B":
                    jA, sA, jppA, segsA, obA, stA, QAa, _PA = st["A"]
                    Wc = jppA + jpp
                    svc = st["sv"][:, :].rearrange("p (q j) -> p q j", q=Wc)[:P]
                    mvc = st["mv"][:, :].rearrange("p (q j) -> p q j", q=Wc)[:P]
                    tmc = st["tm"][:, :].rearrange("p (q j) -> p q j", q=Wc)[:P]
                    lbc = tblt[:P, sA * Wp:(sA + Wc) * Wp].rearrange(
                        "p (q j) -> p q j", q=Wc)
                    nc1P = spoolQ.tile([128, Wc * Wp], f16, tag="nc1P")
                    nc2P = spoolQ.tile([128, Wc * Wp], f16, tag="nc2P")
                    ovP = spoolQ.tile([128, Wc * Wp], f16, tag="ovP")
                    nc1 = nc1P[:, :].rearrange("p (q j) -> p q j", q=Wc)[:P]
                    nc.vector.tensor_tensor(nc1, mvc, lbc, A.is_ge)
                    nc2 = nc2P[:, :].rearrange("p (q j) -> p q j", q=Wc)[:P]
                    nc.vector.tensor_tensor(nc2, svc, tmc, A.is_ge)
                    ovc = ovP[:, :].rearrange("p (q j) -> p q j", q=Wc)[:P]
                    nc.vector.tensor_tensor(ovc, nc1, nc2, A.max)
                    members = [(jA, jppA, segsA, obA, stA, ovc[:, 0:jppA], QAa),
                               (j0, jpp, segs, ob, st_eng, ovc[:, jppA:Wc], QA)]
                else:
                    nc1 = small("nc1", f16)
                    nc.vector.tensor_tensor(nc1, mv, lbv, A.is_ge)
                    nc2 = small("nc2", f16)
                    nc.vector.tensor_tensor(nc2, sv, tm, A.is_ge)
                    ov = small("ov", f16)
                    nc.vector.tensor_tensor(ov, nc1, nc2, A.max)
                    members = [(j0, jpp, segs, ob, st_eng, ov, QA)]

                exp_store(members, P, tail)

            def exp_store(members, P, tail):
                # broadcast multiply + row broadcast expand ov[q, j] (0/1) to
                # the output block [q, 4 rows, j, 2 halves] scaled by 257 into
                # i16 0x0101/0x0000 words.  Two ops, each <= 3 free dims (the
                # Activation ISA rejects higher-rank access patterns).
                e1 = TAILEXP if tail else EXP1
                e2 = TAILEXP if tail else EXP2
                for jm, jppm, segsm, obm, stm, ovm, QAm in members:
                    obvm = obm[:, :].rearrange("p (q r w) -> p q r w",
                                               q=QAm, r=4, w=HJ)
                    for qlo, qhi in segsm:
                        qn = qhi - qlo
                        row0 = obvm[:P, qlo:qhi, 0, :].rearrange(
                            "p q (j k) -> p q j k", j=Wp, k=2)
                        ovb = ovm[:, qlo:qhi].unsqueeze(3).broadcast_to(
                            [P, qn, Wp, 2])
                        rows = obvm[:P, qlo:qhi, 1:4, :]
                        r0b = obvm[:P, qlo:qhi, 0, :].unsqueeze(2).broadcast_to(
                            [P, qn, 3, HJ])
                        if e1 == "act":
                            nc.scalar.mul(row0, ovb, 257.0)
                        elif e1 == "pool":
                            nc.gpsimd.tensor_scalar(row0, ovb, 257.0, None, A.mult)
                        else:
                            nc.vector.tensor_scalar(row0, ovb, 257.0, None, A.mult)
                        if e2 == "act":
                            nc.scalar.copy(rows, r0b)
                        elif e2 == "pool":
                            nc.gpsimd.tensor_copy(rows, r0b)
                        else:
                            nc.vector.tensor_copy(rows, r0b)

                        # dense contiguous store (1264B/job) on the other ring
                        stm.dma_start(
                            out_d[(jm + qlo * P) * OBLK:
                                  (jm + qhi * P) * OBLK].rearrange(
                                "(q p f) -> p q f", q=qn, p=P, f=OBLK),
                            obm[:P, qlo * OBLK:qhi * OBLK].rearrange(
                                "p (q f) -> p q f", q=qn, f=OBLK))

            def finish_solo(st, tail=False):
                """Emit the deferred conds + expansion + store of a stashed
                tile (pair role A with no B)."""
                jA, sA, jppA, segsA, obA, stA, QAa, P = st["A"]
                spoolQ = st["pool"]
                sfx = st["sfx"]
                Wc = jppA
                svc = st["sv"][:, :].rearrange("p (q j) -> p q j", q=Wc)[:P]
                mvc = st["mv"][:, :].rearrange("p (q j) -> p q j", q=Wc)[:P]
                tmc = st["tm"][:, :].rearrange("p (q j) -> p q j", q=Wc)[:P]
                lbc = tblt[:P, sA * Wp:(sA + Wc) * Wp].rearrange(
                    "p (q j) -> p q j", q=Wc)
                nc1P = spoolQ.tile([128, Wc * Wp], f16, tag="nc1P" + sfx)
                nc2P = spoolQ.tile([128, Wc * Wp], f16, tag="nc2P" + sfx)
                ovP = spoolQ.tile([128, Wc * Wp], f16, tag="ovP" + sfx)
                nc1 = nc1P[:, :].rearrange("p (q j) -> p q j", q=Wc)[:P]
                nc.vector.tensor_tensor(nc1, mvc, lbc, A.is_ge)
                nc2 = nc2P[:, :].rearrange("p (q j) -> p q j", q=Wc)[:P]
                nc.vector.tensor_tensor(nc2, svc, tmc, A.is_ge)
                ovc = ovP[:, :].rearrange("p (q j) -> p q j", q=Wc)[:P]
                nc.vector.tensor_tensor(ovc, nc1, nc2, A.max)
                exp_store([(jA, jppA, segsA, obA, stA, ovc, QAa)], P, tail)

            # job-space offsets per tile
            offs = []
            j0 = 0
            s0 = 0
            for q_n, P in TILES:
                offs.append((j0, s0, P, q_n))
                j0 += q_n * P
                s0 += q_n
            n = len(TILES)
            # emission order: t0, t1, then the last NHOIST job-tiles (their
            # stores are ready early and fill DMA gaps near the end), then
            # the remaining middle tiles in job order
            if ORDER_OVERRIDE is not None:
                order = list(ORDER_OVERRIDE)
            else:
                order = [0, 1] + list(range(n - 1, n - 1 - NHOIST, -1)) + \
                    list(range(2, n - NHOIST))
            assert sorted(order) == list(range(n))
            # pre-emit the first TBL_POS tiles' loads so the table DMAs queue
            # behind them on the ring, then emit tables BEFORE any compute
            # that reads them (no use-before-def)
            pre = {}
            for ei in range(min(TBL_POS, n)):
                ti = order[ei]
                j0, s0, P, q_n = offs[ti]
                hoisted = ti >= n - NHOIST
                sfx = f"_h{ti}" if hoisted else ""
                QA = q_n if sfx else JPP
                xpoolQ = hpool if sfx else xpool
                xt = xpoolQ.tile([128, QA * BLK], f16, tag="xt" + sfx)
                ring = nc.gpsimd if ei < LD0_POOL else None
                for qlo, qhi in ([(0, 2), (2, 4)]
                                 if (q_n == 4 and SPLIT_IO) else [(0, q_n)]):
                    emit_load(j0 + qlo * P, P, qhi - qlo, xt, qoff=qlo,
                              ring=ring)
                pre[ti] = xt
            emit_tables()
            # pair adjacent plain middle tiles (same jpp, contiguous slots)
            # so their 79-wide condition ops run as single instructions
            pair_of = {}
            if PAIR_CONDS:
                ei = 0
                while ei < len(order) - 1:
                    ta, tb = order[ei], order[ei + 1]
                    ok = (ta < n - NHOIST and tb < n - NHOIST
                          and ei + 1 < n - NTAIL
                          and offs[ta][3] == offs[tb][3]
                          and offs[ta][2] == offs[tb][2] == 128
                          and offs[tb][1] == offs[ta][1] + offs[ta][3])
                    if ok:
                        stt = {}
                        pair_of[ta] = ("A", stt)
                        pair_of[tb] = ("B", stt)
                        ei += 2
                    else:
                        ei += 1
            deferred = []
            for ei, ti in enumerate(order):
                if ei == FLUSH_AT and deferred:
                    for st_ in deferred:
                        finish_solo(st_)
                    deferred = []
                j0, s0, P, q_n = offs[ti]
                hoisted = ti >= n - NHOIST
                tail = ei >= n - NTAIL
                pairing = pair_of.get(ti)
                if pairing is None and ei < DEFER_N and not tail:
                    st_ = {"solo": True}
                    pairing = ("A", st_)
                    deferred.append(st_)
                do_tile(j0, s0, P, q_n, xt=pre.get(ti),
                        last=ei >= n - 2,
                        tail=tail,
                        sfx=f"_h{ti}" if hoisted else "",
                        pair=pairing)
                del j0, s0
            for st_ in deferred:
                finish_solo(st_)

    nc.compile()
    return nc


def get_nc():
    if "nc" not in _CACHE:
        _CACHE["nc"] = _build_nc()
    return _CACHE["nc"]


def _check_maps(map_rows, map_cols):
    """The device program hardcodes the clip(4i-1..4i+2) scatter footprint;
    verify the provided maps match it exactly."""
    off = np.arange(4)
    rows = np.clip(4 * np.arange(Hp)[:, None] - 1 + off[None, :], 0, H - 1)
    cols = np.clip(4 * np.arange(Wp)[:, None] - 1 + off[None, :], 0, W - 1)
    exp_rows = np.broadcast_to(rows[:, None, :, None], (Hp, Wp, 4, 4)).reshape(Hp, Wp, 16)
    exp_cols = np.broadcast_to(cols[None, :, None, :], (Hp, Wp, 4, 4)).reshape(Hp, Wp, 16)
    if not (np.asarray(map_rows) == exp_rows).all() or \
       not (np.asarray(map_cols) == exp_cols).all():
        raise ValueError("map_rows/map_cols do not match the expected "
                         "clip(4i-1..4i+2) footprint this kernel hardcodes")


def pad_input(x):
    """[n,1,H,W] (or [n,H,W]) f32 -> flat fp16 [n*H2*W2] with a zero ring."""
    if x.ndim == 4:
        x = x[:, 0]
    xp = np.zeros((x.shape[0], H2, W2), np.float16)
    xp[:, 1:H + 1, 1:W + 1] = x.astype(np.float16)
    return np.ascontiguousarray(xp.reshape(-1))


def kernel(x, lower_bound1, q1, map_rows, map_cols):
    from concourse.bass_utils import run_bass_kernel_spmd

    x = np.asarray(x, dtype=np.float32)
    lb = np.ascontiguousarray(np.asarray(lower_bound1, dtype=np.float32))
    q1 = np.ascontiguousarray(np.asarray(q1, dtype=np.float32))
    _check_maps(map_rows, map_cols)
    assert x.shape == (B, 1, H, W), x.shape

    thr4 = (np.float32(4.0) * (q1 / lb).astype(np.float32)).astype(np.float32)
    tbl = np.concatenate(
        [_job_slot_table(lb.astype(np.float16)),
         _job_slot_table(np.clip(thr4, -60000.0, 60000.0).astype(np.float16))],
        axis=1)

    nc = get_nc()
    in_maps = [
        {"xp": pad_input(x[c * BC:(c + 1) * BC]), "tbl": tbl}
        for c in range(NCORES)
    ]
    res = run_bass_kernel_spmd(nc, in_maps, list(range(NCORES)))
    out = np.concatenate(
        [r["out"].view(np.uint8).reshape(BC, H2, W2)[:, 1:H + 1, 1:W + 1]
         for r in res.results],
        axis=0)
    return np.ascontiguousarray(out.reshape(B, 1, H, W).astype(np.float32))
